# revision 4
# baseline (speedup 1.0000x reference)
"""Fused multi-core attention kernel for Trainium2 (Bass/Tile).

Problem: BasicAttention block on x[4, 256, 64, 64]:
    q = Wq x + bq ; k = Wk x + bk ; v = Wv x + bv   (1x1 convs)
    energy = q^T k * IC^-0.5 ; attn = softmax(energy, keys)
    out = gamma * (v @ attn^T) + 2 x

Sharding: 8 cores = (batch b in 0..3) x (query-row half r in 0..1).
Each core computes a [C=256, 2048] slice of the output for batch b,
pixel rows r*2048..(r+1)*2048, flash-attention style (energy never
leaves SBUF). All heavy matmuls run in bf16 (output is dominated by
the exact-f32 2x term; attention contributes ~2.5e-4 of magnitude).

Per-core dataflow (N=4096 keys, ROWS=2048 queries, IC=128):
  K  [128,4096]  = WkT.T @ Xbf16        (+bk)
  Q  [128,2048]  = WqT.T @ XRbf16       (+bq)
  VT [4096, 256] = X.T @ WvT (+bv via k=1 ones matmul), tiled [128,32,256]
  per 512-query chunk:
    E^T tile [128m, 512n] = K_mb.T @ Q_chunk  -> PSUM
    P^T = exp(scale * E^T) -> bf16 SBUF       (no max-sub: |scaled E|<~1)
    U[c,n] += VT_mb.T @ P^T_mb                (PSUM accum, c=2x128)
    S[n] = ones.T @ (sum_mb P^T)              (DVE reduce + 1 matmul)
    y = gamma*U/S + 2x                        (DVE, f32)
"""

import os
import sys

for _p in ("/opt/trn_rl_repo", "/root/.axon_site/_ro/trn_rl_repo"):
    if os.path.isdir(_p) and _p not in sys.path:
        sys.path.append(_p)

import numpy as np
import ml_dtypes

import concourse.bass as bass
import concourse.mybir as mybir
import concourse.tile as tile
from concourse.bass_utils import run_bass_kernel_spmd

BF16 = mybir.dt.bfloat16
F32 = mybir.dt.float32
NPBF16 = ml_dtypes.bfloat16

B, C, H, W = 4, 256, 64, 64
N = H * W              # 4096 pixels (keys)
IC = C // 2            # 128 inter channels
NCORES = 8
ROWS = N * B // NCORES  # 2048 query rows per core
CHUNK = 512            # query rows per softmax chunk
NCH = ROWS // CHUNK    # 4 chunks
MB = N // 128          # 32 key blocks
SCALE = float(IC) ** -0.5


def _split_waits(nc):
    """This container's walrus accepts only ONE sync-wait per instruction.
    Hoist extra waits onto single-wait NOPs inserted just before the
    instruction on the same engine (identical stall semantics)."""
    for f in nc.m.functions:
        for b in f.blocks:
            insts = b.instructions
            i = 0
            while i < len(insts):
                inst = insts[i]
                si = inst.sync_info
                if si is not None and len(si.on_wait) > 1:
                    waits = list(si.on_wait)
                    si.on_wait = waits[-1:]
                    for w in waits[:-1]:
                        nop = mybir.InstNoOp(
                            name=f"I-wsplit-{nc.next_id()}",
                            engine=inst.engine,
                            ins=[],
                            outs=[],
                            sync_info=mybir.SyncInfo(on_wait=[w], on_update=[]),
                        )
                        insts.insert(i, nop)
                        i += 1
                i += 1


def _build():
    nc = bass.Bass()

    xf_d = nc.dram_tensor("xf", [C, N], F32, kind="ExternalInput")
    xr_d = nc.dram_tensor("xr", [C, ROWS], F32, kind="ExternalInput")
    wqT_d = nc.dram_tensor("wqT", [C, IC], BF16, kind="ExternalInput")
    wkT_d = nc.dram_tensor("wkT", [C, IC], BF16, kind="ExternalInput")
    wvT_d = nc.dram_tensor("wvT", [C, C], BF16, kind="ExternalInput")
    bq_d = nc.dram_tensor("bq", [IC, 1], F32, kind="ExternalInput")
    bk_d = nc.dram_tensor("bk", [IC, 1], F32, kind="ExternalInput")
    bvr_d = nc.dram_tensor("bvr", [1, C], BF16, kind="ExternalInput")
    gamma_d = nc.dram_tensor("gamma", [1, 1], F32, kind="ExternalInput")
    y_d = nc.dram_tensor("y", [C, ROWS], F32, kind="ExternalOutput")

    with tile.TileContext(nc) as tc:
        with (
            tc.tile_pool(name="consts", bufs=1) as consts,
            tc.tile_pool(name="xf", bufs=2) as xfp,
            tc.tile_pool(name="xb", bufs=2) as xbp,
            tc.tile_pool(name="xr", bufs=2) as xrp,
            tc.tile_pool(name="xrb", bufs=2) as xrbp,
            tc.tile_pool(name="kq", bufs=1) as kqp,
            tc.tile_pool(name="vt", bufs=1) as vtp,
            tc.tile_pool(name="pt", bufs=1) as ptp,
            tc.tile_pool(name="sm", bufs=2) as smp,
            tc.tile_pool(name="outp", bufs=4) as outp,
            tc.tile_pool(name="eg", bufs=2, space="PSUM") as egp,
            tc.tile_pool(name="up", bufs=1, space="PSUM") as upp,
            tc.tile_pool(name="sp", bufs=1, space="PSUM") as spp,
            tc.tile_pool(name="bc", bufs=1, space="PSUM") as bcp,
        ):
            # ---- constants ----
            wqT = consts.tile([128, 2, IC], BF16, tag="wqT")
            nc.gpsimd.dma_start(out=wqT, in_=wqT_d.rearrange("(t p) o -> p t o", p=128))
            wkT = consts.tile([128, 2, IC], BF16, tag="wkT")
            nc.gpsimd.dma_start(out=wkT, in_=wkT_d.rearrange("(t p) o -> p t o", p=128))
            wvT = consts.tile([128, 2, C], BF16, tag="wvT")
            nc.gpsimd.dma_start(out=wvT, in_=wvT_d.rearrange("(t p) o -> p t o", p=128))
            bq = consts.tile([IC, 1], F32, tag="bq")
            nc.gpsimd.dma_start(out=bq, in_=bq_d[:])
            bk = consts.tile([IC, 1], F32, tag="bk")
            nc.gpsimd.dma_start(out=bk, in_=bk_d[:])
            bvr = consts.tile([1, C], BF16, tag="bvr")
            nc.gpsimd.dma_start(out=bvr, in_=bvr_d[:])
            gamma = consts.tile([1, 1], F32, tag="gamma")
            nc.gpsimd.dma_start(out=gamma, in_=gamma_d[:])
            ones_bf_row = consts.tile([1, 128], BF16, tag="ones_bf_row")
            nc.vector.memset(ones_bf_row, 1.0)
            ones_f_col = consts.tile([128, 1], F32, tag="ones_f_col")
            nc.vector.memset(ones_f_col, 1.0)
            ones_f_row = consts.tile([1, 128], F32, tag="ones_f_row")
            nc.vector.memset(ones_f_row, 1.0)

            # ---- load x, convert to bf16 ----
            xf = []
            xb = []
            for ci in range(2):
                t = xfp.tile([128, N], F32, tag="xf")
                nc.gpsimd.dma_start(out=t, in_=xf_d[ci * 128 : (ci + 1) * 128, :])
                xf.append(t)
                tb = xbp.tile([128, N], BF16, tag="xb")
                nc.vector.tensor_copy(tb, t)
                xb.append(tb)
            xr = []
            xrb = []
            for ci in range(2):
                t = xrp.tile([128, ROWS], F32, tag="xr")
                nc.gpsimd.dma_start(out=t, in_=xr_d[ci * 128 : (ci + 1) * 128, :])
                xr.append(t)
                tb = xrbp.tile([128, ROWS], BF16, tag="xrb")
                nc.vector.tensor_copy(tb, t)
                xrb.append(tb)

            # ---- K = WkT.T @ X (+bk), Q = WqT.T @ XR (+bq) ----
            kbuf = kqp.tile([128, N], BF16, tag="kbuf")
            for nt in range(N // 512):
                ps = egp.tile([128, 512], F32, tag="eg")
                for ci in range(2):
                    nc.tensor.matmul(
                        ps,
                        wkT[:, ci, :],
                        xb[ci][:, nt * 512 : (nt + 1) * 512],
                        start=(ci == 0),
                        stop=(ci == 1),
                    )
                nc.vector.tensor_scalar_add(kbuf[:, nt * 512 : (nt + 1) * 512], ps, bk)
            qbuf = kqp.tile([128, ROWS], BF16, tag="qbuf")
            for nt in range(ROWS // 512):
                ps = egp.tile([128, 512], F32, tag="eg")
                for ci in range(2):
                    nc.tensor.matmul(
                        ps,
                        wqT[:, ci, :],
                        xrb[ci][:, nt * 512 : (nt + 1) * 512],
                        start=(ci == 0),
                        stop=(ci == 1),
                    )
                nc.vector.tensor_scalar_add(qbuf[:, nt * 512 : (nt + 1) * 512], ps, bq)

            # ---- VT[m, c] = X.T @ WvT + bv ----
            vt = vtp.tile([128, MB, C], BF16, tag="vt")
            for mb in range(MB):
                ps = egp.tile([128, C], F32, tag="eg")
                for ci in range(2):
                    nc.tensor.matmul(
                        ps,
                        xb[ci][:, mb * 128 : (mb + 1) * 128],
                        wvT[:, ci, :],
                        start=(ci == 0),
                        stop=False,
                    )
                nc.tensor.matmul(ps, ones_bf_row, bvr, start=False, stop=True)
                nc.scalar.copy(vt[:, mb, :], ps)

            # ---- attention main loop ----
            for ch in range(NCH):
                qs = qbuf[:, ch * CHUNK : (ch + 1) * CHUNK]
                ptb = ptp.tile([128, MB, CHUNK], BF16, tag="pt")
                u01 = [
                    upp.tile([128, CHUNK], F32, tag="u0", name="u0"),
                    upp.tile([128, CHUNK], F32, tag="u1", name="u1"),
                ]
                for g in range(MB // 2):
                    eg = egp.tile([128, 2, CHUNK], F32, tag="eg")
                    for j in range(2):
                        mb = 2 * g + j
                        nc.tensor.matmul(
                            eg[:, j, :],
                            kbuf[:, mb * 128 : (mb + 1) * 128],
                            qs,
                            start=True,
                            stop=True,
                        )
                    nc.scalar.activation(
                        ptb[:, 2 * g : 2 * g + 2, :],
                        eg,
                        mybir.ActivationFunctionType.Exp,
                        scale=SCALE,
                    )
                    for j in range(2):
                        mb = 2 * g + j
                        for cc in range(2):
                            nc.tensor.matmul(
                                u01[cc],
                                vt[:, mb, cc * 128 : (cc + 1) * 128],
                                ptb[:, mb, :],
                                start=(mb == 0),
                                stop=(mb == MB - 1),
                            )
                # row sums S[n] = sum_m P^T[m, n]
                tsum = smp.tile([128, CHUNK], F32, tag="tsum")
                nc.vector.reduce_sum(
                    tsum,
                    ptb.rearrange("p m n -> p n m"),
                    axis=mybir.AxisListType.X,
                )
                s_ps = spp.tile([1, CHUNK], F32, tag="s")
                nc.tensor.matmul(s_ps, ones_f_col, tsum, start=True, stop=True)
                sinv = smp.tile([1, CHUNK], F32, tag="sinv")
                nc.vector.reciprocal(sinv, s_ps)
                sg = smp.tile([1, CHUNK], F32, tag="sg")
                nc.vector.tensor_scalar_mul(sg, sinv, gamma[0:1, 0:1])
                # broadcast gamma/S across partitions via k=1 matmul
                sgb_ps = bcp.tile([128, CHUNK], F32, tag="sgb")
                nc.tensor.matmul(sgb_ps, ones_f_row, sg, start=True, stop=True)
                sgb = smp.tile([128, CHUNK], F32, tag="sgbs")
                nc.vector.tensor_copy(sgb, sgb_ps)
                # y = (U * gamma/S) + 2*x
                for cc in range(2):
                    tmp = outp.tile([128, CHUNK], F32, tag="tmp")
                    nc.vector.tensor_tensor(tmp, u01[cc], sgb, op=mybir.AluOpType.mult)
                    out_t = outp.tile([128, CHUNK], F32, tag="out")
                    nc.vector.scalar_tensor_tensor(
                        out_t,
                        xr[cc][:, ch * CHUNK : (ch + 1) * CHUNK],
                        2.0,
                        tmp,
                        op0=mybir.AluOpType.mult,
                        op1=mybir.AluOpType.add,
                    )
                    nc.gpsimd.dma_start(
                        out=y_d[
                            cc * 128 : (cc + 1) * 128,
                            ch * CHUNK : (ch + 1) * CHUNK,
                        ],
                        in_=out_t,
                    )
    _split_waits(nc)
    return nc


_NC_CACHE = None


def _get_nc():
    global _NC_CACHE
    if _NC_CACHE is None:
        _NC_CACHE = _build()
    return _NC_CACHE


def kernel(x, Wq, bq, Wk, bk, Wv, bv, gamma):
    x = np.asarray(x, dtype=np.float32)
    nc = _get_nc()
    wqT = np.ascontiguousarray(np.asarray(Wq, np.float32).T.astype(NPBF16))
    wkT = np.ascontiguousarray(np.asarray(Wk, np.float32).T.astype(NPBF16))
    wvT = np.ascontiguousarray(np.asarray(Wv, np.float32).T.astype(NPBF16))
    shared = {
        "wqT": wqT,
        "wkT": wkT,
        "wvT": wvT,
        "bq": np.asarray(bq, np.float32).reshape(IC, 1).copy(),
        "bk": np.asarray(bk, np.float32).reshape(IC, 1).copy(),
        "bvr": np.asarray(bv, np.float32).reshape(1, C).astype(NPBF16).copy(),
        "gamma": np.asarray(gamma, np.float32).reshape(1, 1).copy(),
    }
    xflat = x.reshape(B, C, N)
    in_maps = []
    for core in range(NCORES):
        b, r = divmod(core, 2)
        xf = np.ascontiguousarray(xflat[b])
        xr = np.ascontiguousarray(xflat[b][:, r * ROWS : (r + 1) * ROWS])
        in_maps.append({"xf": xf, "xr": xr, **shared})

    trace = bool(int(os.environ.get("KERNEL_TRACE", "0")))
    res = run_bass_kernel_spmd(
        nc, in_maps, core_ids=list(range(NCORES)), trace=trace
    )
    if trace:
        global LAST_RESULT
        LAST_RESULT = res

    out = np.empty((B, C, N), np.float32)
    for core in range(NCORES):
        b, r = divmod(core, 2)
        out[b][:, r * ROWS : (r + 1) * ROWS] = res.results[core]["y"]
    return out.reshape(B, C, H, W)


if __name__ == "__main__":
    rng = np.random.default_rng(0)
    x = rng.standard_normal((B, C, H, W), dtype=np.float32)
    s = 0.02
    out = kernel(
        x=x,
        Wq=(rng.standard_normal((IC, C)) * s).astype(np.float32),
        bq=np.zeros(IC, np.float32),
        Wk=(rng.standard_normal((IC, C)) * s).astype(np.float32),
        bk=np.zeros(IC, np.float32),
        Wv=(rng.standard_normal((C, C)) * s).astype(np.float32),
        bv=np.zeros(C, np.float32),
        gamma=np.full(1, 0.1, np.float32),
    )
    print("out", out.shape, out.dtype, float(out.ravel()[0]))


# revision 7
# speedup vs baseline: 1.2480x; 1.2480x over previous
"""Fused multi-core attention kernel for Trainium2 (Bass/Tile).

Problem: BasicAttention block on x[4, 256, 64, 64]:
    q = Wq x + bq ; k = Wk x + bk ; v = Wv x + bv   (1x1 convs)
    energy = q^T k * IC^-0.5 ; attn = softmax(energy, keys)
    out = gamma * (v @ attn^T) + 2 x

Sharding: 8 cores = (batch b in 0..3) x (query-row half r in 0..1).
Each core computes a [C=256, 2048] slice of the output for batch b,
pixel rows r*2048..(r+1)*2048, flash-attention style (energy never
leaves SBUF). All heavy matmuls run in bf16 (output is dominated by
the exact-f32 2x term; attention contributes ~2.5e-4 of magnitude).

Per-core dataflow (N=4096 keys, ROWS=2048 queries, IC=128):
  K  [128,4096]  = WkT.T @ Xbf16        (+bk)
  Q  [128,2048]  = WqT.T @ XRbf16       (+bq)
  VT [4096, 256] = X.T @ WvT (+bv via k=1 ones matmul), tiled [128,32,256]
  per 512-query chunk:
    E^T tile [128m, 512n] = K_mb.T @ Q_chunk  -> PSUM
    P^T = exp(scale * E^T) -> bf16 SBUF       (no max-sub: |scaled E|<~1)
    U[c,n] += VT_mb.T @ P^T_mb                (PSUM accum, c=2x128)
    S[n] = ones.T @ (sum_mb P^T)              (DVE reduce + 1 matmul)
    y = gamma*U/S + 2x                        (DVE, f32)
"""

import os
import sys

for _p in ("/opt/trn_rl_repo", "/root/.axon_site/_ro/trn_rl_repo"):
    if os.path.isdir(_p) and _p not in sys.path:
        sys.path.append(_p)

import numpy as np
import ml_dtypes

import concourse.bass as bass
import concourse.mybir as mybir
import concourse.tile as tile
from concourse.bass_utils import run_bass_kernel_spmd

BF16 = mybir.dt.bfloat16
F32 = mybir.dt.float32
NPBF16 = ml_dtypes.bfloat16

B, C, H, W = 4, 256, 64, 64
N = H * W              # 4096 pixels (keys)
IC = C // 2            # 128 inter channels
NCORES = 8
ROWS = N * B // NCORES  # 2048 query rows per core
CHUNK = 512            # query rows per softmax chunk
NCH = ROWS // CHUNK    # 4 chunks
MB = N // 128          # 32 key blocks
SCALE = float(IC) ** -0.5


def _split_waits(nc):
    """This container's walrus accepts only ONE sync-wait per instruction.
    Hoist extra waits onto single-wait NOPs inserted just before the
    instruction on the same engine (identical stall semantics)."""
    for f in nc.m.functions:
        for b in f.blocks:
            insts = b.instructions
            i = 0
            while i < len(insts):
                inst = insts[i]
                si = inst.sync_info
                if si is not None and len(si.on_wait) > 1:
                    waits = list(si.on_wait)
                    si.on_wait = waits[-1:]
                    for w in waits[:-1]:
                        nop = mybir.InstNoOp(
                            name=f"I-wsplit-{nc.next_id()}",
                            engine=inst.engine,
                            ins=[],
                            outs=[],
                            sync_info=mybir.SyncInfo(on_wait=[w], on_update=[]),
                        )
                        insts.insert(i, nop)
                        i += 1
                i += 1


def _build():
    nc = bass.Bass()

    xf_d = nc.dram_tensor("xf", [C, N], F32, kind="ExternalInput")
    xr_d = nc.dram_tensor("xr", [C, ROWS], F32, kind="ExternalInput")
    wqT_d = nc.dram_tensor("wqT", [C, IC], BF16, kind="ExternalInput")
    wkT_d = nc.dram_tensor("wkT", [C, IC], BF16, kind="ExternalInput")
    wvT_d = nc.dram_tensor("wvT", [C, C], BF16, kind="ExternalInput")
    bq_d = nc.dram_tensor("bq", [IC, 1], F32, kind="ExternalInput")
    bk_d = nc.dram_tensor("bk", [IC, 1], F32, kind="ExternalInput")
    bvr_d = nc.dram_tensor("bvr", [1, C], BF16, kind="ExternalInput")
    gamma_d = nc.dram_tensor("gamma", [1, 1], F32, kind="ExternalInput")
    y_d = nc.dram_tensor("y", [C, ROWS], F32, kind="ExternalOutput")

    with tile.TileContext(nc) as tc:
        with (
            tc.tile_pool(name="consts", bufs=1) as consts,
            tc.tile_pool(name="xf", bufs=2) as xfp,
            tc.tile_pool(name="xb", bufs=2) as xbp,
            tc.tile_pool(name="xr", bufs=2) as xrp,
            tc.tile_pool(name="xrb", bufs=2) as xrbp,
            tc.tile_pool(name="kq", bufs=1) as kqp,
            tc.tile_pool(name="vt", bufs=1) as vtp,
            tc.tile_pool(name="pt", bufs=2) as ptp,
            tc.tile_pool(name="sm", bufs=2) as smp,
            tc.tile_pool(name="outp", bufs=4) as outp,
            tc.tile_pool(name="eg", bufs=2, space="PSUM") as egp,
            tc.tile_pool(name="up", bufs=1, space="PSUM") as upp,
            tc.tile_pool(name="sp", bufs=1, space="PSUM") as spp,
            tc.tile_pool(name="bc", bufs=1, space="PSUM") as bcp,
        ):
            # ---- constants ----
            wqT = consts.tile([128, 2, IC], BF16, tag="wqT")
            nc.gpsimd.dma_start(out=wqT, in_=wqT_d.rearrange("(t p) o -> p t o", p=128))
            wkT = consts.tile([128, 2, IC], BF16, tag="wkT")
            nc.gpsimd.dma_start(out=wkT, in_=wkT_d.rearrange("(t p) o -> p t o", p=128))
            wvT = consts.tile([128, 2, C], BF16, tag="wvT")
            nc.gpsimd.dma_start(out=wvT, in_=wvT_d.rearrange("(t p) o -> p t o", p=128))
            bq = consts.tile([IC, 1], F32, tag="bq")
            nc.gpsimd.dma_start(out=bq, in_=bq_d[:])
            bk = consts.tile([IC, 1], F32, tag="bk")
            nc.gpsimd.dma_start(out=bk, in_=bk_d[:])
            bvr = consts.tile([1, C], BF16, tag="bvr")
            nc.gpsimd.dma_start(out=bvr, in_=bvr_d[:])
            gamma = consts.tile([1, 1], F32, tag="gamma")
            nc.gpsimd.dma_start(out=gamma, in_=gamma_d[:])
            ones_bf_row = consts.tile([1, 128], BF16, tag="ones_bf_row")
            nc.vector.memset(ones_bf_row, 1.0)
            ones_bf_col = consts.tile([128, 1], BF16, tag="ones_bf_col")
            nc.vector.memset(ones_bf_col, 1.0)
            ones_f_row = consts.tile([1, 128], F32, tag="ones_f_row")
            nc.vector.memset(ones_f_row, 1.0)

            # ---- load x, convert to bf16 ----
            xf = []
            xb = []
            for ci in range(2):
                t = xfp.tile([128, N], F32, tag="xf")
                nc.gpsimd.dma_start(out=t, in_=xf_d[ci * 128 : (ci + 1) * 128, :])
                xf.append(t)
                tb = xbp.tile([128, N], BF16, tag="xb")
                nc.vector.tensor_copy(tb, t)
                xb.append(tb)
            xr = []
            xrb = []
            for ci in range(2):
                t = xrp.tile([128, ROWS], F32, tag="xr")
                nc.gpsimd.dma_start(out=t, in_=xr_d[ci * 128 : (ci + 1) * 128, :])
                xr.append(t)
                tb = xrbp.tile([128, ROWS], BF16, tag="xrb")
                nc.vector.tensor_copy(tb, t)
                xrb.append(tb)

            # ---- K = WkT.T @ X (+bk), Q = WqT.T @ XR (+bq) ----
            kbuf = kqp.tile([128, N], BF16, tag="kbuf")
            for nt in range(N // 512):
                ps = egp.tile([128, 512], F32, tag="eg")
                for ci in range(2):
                    nc.tensor.matmul(
                        ps,
                        wkT[:, ci, :],
                        xb[ci][:, nt * 512 : (nt + 1) * 512],
                        start=(ci == 0),
                        stop=(ci == 1),
                    )
                nc.vector.tensor_scalar_add(kbuf[:, nt * 512 : (nt + 1) * 512], ps, bk)
            qbuf = kqp.tile([128, ROWS], BF16, tag="qbuf")
            for nt in range(ROWS // 512):
                ps = egp.tile([128, 512], F32, tag="eg")
                for ci in range(2):
                    nc.tensor.matmul(
                        ps,
                        wqT[:, ci, :],
                        xrb[ci][:, nt * 512 : (nt + 1) * 512],
                        start=(ci == 0),
                        stop=(ci == 1),
                    )
                nc.vector.tensor_scalar_add(qbuf[:, nt * 512 : (nt + 1) * 512], ps, bq)

            # ---- VT[m, c] = X.T @ WvT + bv ----
            vt = vtp.tile([128, MB, C], BF16, tag="vt")
            for mb in range(MB):
                ps = egp.tile([128, C], F32, tag="eg")
                for ci in range(2):
                    nc.tensor.matmul(
                        ps,
                        xb[ci][:, mb * 128 : (mb + 1) * 128],
                        wvT[:, ci, :],
                        start=(ci == 0),
                        stop=False,
                    )
                nc.tensor.matmul(ps, ones_bf_row, bvr, start=False, stop=True)
                nc.scalar.copy(vt[:, mb, :], ps)

            # ---- attention main loop ----
            for ch in range(NCH):
                qs = qbuf[:, ch * CHUNK : (ch + 1) * CHUNK]
                ptb = ptp.tile([128, MB, CHUNK], BF16, tag="pt")
                u01 = [
                    upp.tile([128, CHUNK], F32, tag="u0", name="u0"),
                    upp.tile([128, CHUNK], F32, tag="u1", name="u1"),
                ]
                s_ps = spp.tile([1, CHUNK], F32, tag="s")
                for g in range(MB // 2):
                    eg = egp.tile([128, 2, CHUNK], F32, tag="eg")
                    for j in range(2):
                        mb = 2 * g + j
                        nc.tensor.matmul(
                            eg[:, j, :],
                            kbuf[:, mb * 128 : (mb + 1) * 128],
                            qs,
                            start=True,
                            stop=True,
                        )
                    nc.scalar.activation(
                        ptb[:, 2 * g : 2 * g + 2, :],
                        eg,
                        mybir.ActivationFunctionType.Exp,
                        scale=SCALE,
                    )
                    for j in range(2):
                        mb = 2 * g + j
                        # row sums S[n] += ones.T @ P^T_mb (PSUM accumulate)
                        nc.tensor.matmul(
                            s_ps,
                            ones_bf_col,
                            ptb[:, mb, :],
                            start=(mb == 0),
                            stop=(mb == MB - 1),
                        )
                        for cc in range(2):
                            nc.tensor.matmul(
                                u01[cc],
                                vt[:, mb, cc * 128 : (cc + 1) * 128],
                                ptb[:, mb, :],
                                start=(mb == 0),
                                stop=(mb == MB - 1),
                            )
                sinv = smp.tile([1, CHUNK], F32, tag="sinv")
                nc.vector.reciprocal(sinv, s_ps)
                sg = smp.tile([1, CHUNK], F32, tag="sg")
                nc.vector.tensor_scalar_mul(sg, sinv, gamma[0:1, 0:1])
                # broadcast gamma/S across partitions via k=1 matmul
                sgb_ps = bcp.tile([128, CHUNK], F32, tag="sgb")
                nc.tensor.matmul(sgb_ps, ones_f_row, sg, start=True, stop=True)
                sgb = smp.tile([128, CHUNK], F32, tag="sgbs")
                nc.vector.tensor_copy(sgb, sgb_ps)
                # y = (U * gamma/S) + 2*x
                for cc in range(2):
                    tmp = outp.tile([128, CHUNK], F32, tag="tmp")
                    nc.vector.tensor_tensor(tmp, u01[cc], sgb, op=mybir.AluOpType.mult)
                    out_t = outp.tile([128, CHUNK], F32, tag="out")
                    nc.vector.scalar_tensor_tensor(
                        out_t,
                        xr[cc][:, ch * CHUNK : (ch + 1) * CHUNK],
                        2.0,
                        tmp,
                        op0=mybir.AluOpType.mult,
                        op1=mybir.AluOpType.add,
                    )
                    nc.gpsimd.dma_start(
                        out=y_d[
                            cc * 128 : (cc + 1) * 128,
                            ch * CHUNK : (ch + 1) * CHUNK,
                        ],
                        in_=out_t,
                    )
    _split_waits(nc)
    return nc


_NC_CACHE = None


def _get_nc():
    global _NC_CACHE
    if _NC_CACHE is None:
        _NC_CACHE = _build()
    return _NC_CACHE


def kernel(x, Wq, bq, Wk, bk, Wv, bv, gamma):
    x = np.asarray(x, dtype=np.float32)
    nc = _get_nc()
    wqT = np.ascontiguousarray(np.asarray(Wq, np.float32).T.astype(NPBF16))
    wkT = np.ascontiguousarray(np.asarray(Wk, np.float32).T.astype(NPBF16))
    wvT = np.ascontiguousarray(np.asarray(Wv, np.float32).T.astype(NPBF16))
    shared = {
        "wqT": wqT,
        "wkT": wkT,
        "wvT": wvT,
        "bq": np.asarray(bq, np.float32).reshape(IC, 1).copy(),
        "bk": np.asarray(bk, np.float32).reshape(IC, 1).copy(),
        "bvr": np.asarray(bv, np.float32).reshape(1, C).astype(NPBF16).copy(),
        "gamma": np.asarray(gamma, np.float32).reshape(1, 1).copy(),
    }
    xflat = x.reshape(B, C, N)
    in_maps = []
    for core in range(NCORES):
        b, r = divmod(core, 2)
        xf = np.ascontiguousarray(xflat[b])
        xr = np.ascontiguousarray(xflat[b][:, r * ROWS : (r + 1) * ROWS])
        in_maps.append({"xf": xf, "xr": xr, **shared})

    trace = bool(int(os.environ.get("KERNEL_TRACE", "0")))
    res = run_bass_kernel_spmd(
        nc, in_maps, core_ids=list(range(NCORES)), trace=trace
    )
    if trace:
        global LAST_RESULT
        LAST_RESULT = res

    out = np.empty((B, C, N), np.float32)
    for core in range(NCORES):
        b, r = divmod(core, 2)
        out[b][:, r * ROWS : (r + 1) * ROWS] = res.results[core]["y"]
    return out.reshape(B, C, H, W)


if __name__ == "__main__":
    rng = np.random.default_rng(0)
    x = rng.standard_normal((B, C, H, W), dtype=np.float32)
    s = 0.02
    out = kernel(
        x=x,
        Wq=(rng.standard_normal((IC, C)) * s).astype(np.float32),
        bq=np.zeros(IC, np.float32),
        Wk=(rng.standard_normal((IC, C)) * s).astype(np.float32),
        bk=np.zeros(IC, np.float32),
        Wv=(rng.standard_normal((C, C)) * s).astype(np.float32),
        bv=np.zeros(C, np.float32),
        gamma=np.full(1, 0.1, np.float32),
    )
    print("out", out.shape, out.dtype, float(out.ravel()[0]))


# revision 10
# speedup vs baseline: 1.8273x; 1.4642x over previous
"""Fused multi-core attention kernel for Trainium2 (Bass/Tile).

Problem: BasicAttention block on x[4, 256, 64, 64]:
    q = Wq x + bq ; k = Wk x + bk ; v = Wv x + bv   (1x1 convs)
    energy = q^T k * IC^-0.5 ; attn = softmax(energy, keys)
    out = gamma * (v @ attn^T) + 2 x

Sharding: 8 cores = (batch b in 0..3) x (query-row half r in 0..1).
Each core computes a [C=256, 2048] slice of the output for batch b,
pixel rows r*2048..(r+1)*2048, flash-attention style (energy never
leaves SBUF). All heavy matmuls run in bf16 (output is dominated by
the exact-f32 2x term; attention contributes ~2.5e-4 of magnitude).

Per-core dataflow (N=4096 keys, ROWS=2048 queries, IC=128):
  K  [128,4096]  = WkT.T @ Xbf16        (+bk)
  Q  [128,2048]  = WqT.T @ XRbf16       (+bq)
  VT [4096, 256] = X.T @ WvT (+bv via k=1 ones matmul), tiled [128,32,256]
  per 512-query chunk:
    E^T tile [128m, 512n] = K_mb.T @ Q_chunk  -> PSUM
    P^T = exp(scale * E^T) -> bf16 SBUF       (no max-sub: |scaled E|<~1)
    U[c,n] += VT_mb.T @ P^T_mb                (PSUM accum, c=2x128)
    S[n] = ones.T @ (sum_mb P^T)              (DVE reduce + 1 matmul)
    y = gamma*U/S + 2x                        (DVE, f32)
"""

import os
import sys

for _p in ("/opt/trn_rl_repo", "/root/.axon_site/_ro/trn_rl_repo"):
    if os.path.isdir(_p) and _p not in sys.path:
        sys.path.append(_p)

import numpy as np
import ml_dtypes

import concourse.bass as bass
import concourse.mybir as mybir
import concourse.tile as tile
from concourse.bass_utils import run_bass_kernel_spmd

BF16 = mybir.dt.bfloat16
F8 = mybir.dt.float8e4
F32 = mybir.dt.float32
NPBF16 = ml_dtypes.bfloat16

B, C, H, W = 4, 256, 64, 64
N = H * W              # 4096 pixels (keys)
IC = C // 2            # 128 inter channels
NCORES = 8
ROWS = N * B // NCORES  # 2048 query rows per core
CHUNK = 512            # query rows per softmax chunk
NCH = ROWS // CHUNK    # 4 chunks
MB = N // 128          # 32 key blocks
SCALE = float(IC) ** -0.5


def _split_waits(nc):
    """This container's walrus accepts only ONE sync-wait per instruction.
    Hoist extra waits onto single-wait NOPs inserted just before the
    instruction on the same engine (identical stall semantics)."""
    for f in nc.m.functions:
        for b in f.blocks:
            insts = b.instructions
            i = 0
            while i < len(insts):
                inst = insts[i]
                si = inst.sync_info
                if si is not None and len(si.on_wait) > 1:
                    waits = list(si.on_wait)
                    si.on_wait = waits[-1:]
                    for w in waits[:-1]:
                        nop = mybir.InstNoOp(
                            name=f"I-wsplit-{nc.next_id()}",
                            engine=inst.engine,
                            ins=[],
                            outs=[],
                            sync_info=mybir.SyncInfo(on_wait=[w], on_update=[]),
                        )
                        insts.insert(i, nop)
                        i += 1
                i += 1


def _build():
    nc = bass.Bass()

    xf_d = nc.dram_tensor("xf", [C, N], F32, kind="ExternalInput")
    xr_d = nc.dram_tensor("xr", [C, ROWS], F32, kind="ExternalInput")
    wqT_d = nc.dram_tensor("wqT", [C, IC], BF16, kind="ExternalInput")
    wkT_d = nc.dram_tensor("wkT", [C, IC], BF16, kind="ExternalInput")
    wvT_d = nc.dram_tensor("wvT", [C, C], BF16, kind="ExternalInput")
    bq_d = nc.dram_tensor("bq", [IC, 1], F32, kind="ExternalInput")
    bk_d = nc.dram_tensor("bk", [IC, 1], F32, kind="ExternalInput")
    bvr_d = nc.dram_tensor("bvr", [1, C], BF16, kind="ExternalInput")
    gamma_d = nc.dram_tensor("gamma", [1, 1], F32, kind="ExternalInput")
    y_d = nc.dram_tensor("y", [C, ROWS], F32, kind="ExternalOutput")

    with tile.TileContext(nc) as tc:
        with (
            tc.tile_pool(name="consts", bufs=1) as consts,
            tc.tile_pool(name="xf", bufs=2) as xfp,
            tc.tile_pool(name="xb", bufs=2) as xbp,
            tc.tile_pool(name="xr", bufs=2) as xrp,
            tc.tile_pool(name="xrb", bufs=2) as xrbp,
            tc.tile_pool(name="kq", bufs=1) as kqp,
            tc.tile_pool(name="vt", bufs=1) as vtp,
            tc.tile_pool(name="pt", bufs=2) as ptp,
            tc.tile_pool(name="sm", bufs=2) as smp,
            tc.tile_pool(name="outp", bufs=4) as outp,
            tc.tile_pool(name="eg", bufs=2, space="PSUM") as egp,
            tc.tile_pool(name="up", bufs=1, space="PSUM") as upp,
            tc.tile_pool(name="sp", bufs=1, space="PSUM") as spp,
            tc.tile_pool(name="bc", bufs=1, space="PSUM") as bcp,
        ):
            # ---- constants ----
            wqT = consts.tile([128, 2, IC], BF16, tag="wqT")
            nc.gpsimd.dma_start(out=wqT, in_=wqT_d.rearrange("(t p) o -> p t o", p=128))
            wkT = consts.tile([128, 2, IC], BF16, tag="wkT")
            nc.gpsimd.dma_start(out=wkT, in_=wkT_d.rearrange("(t p) o -> p t o", p=128))
            wvT = consts.tile([128, 2, C], BF16, tag="wvT")
            nc.gpsimd.dma_start(out=wvT, in_=wvT_d.rearrange("(t p) o -> p t o", p=128))
            bq = consts.tile([IC, 1], F32, tag="bq")
            nc.gpsimd.dma_start(out=bq, in_=bq_d[:])
            bk = consts.tile([IC, 1], F32, tag="bk")
            nc.gpsimd.dma_start(out=bk, in_=bk_d[:])
            bvr = consts.tile([1, C], BF16, tag="bvr")
            nc.gpsimd.dma_start(out=bvr, in_=bvr_d[:])
            gamma = consts.tile([1, 1], F32, tag="gamma")
            nc.gpsimd.dma_start(out=gamma, in_=gamma_d[:])
            ones_bf_row = consts.tile([1, 128], BF16, tag="ones_bf_row")
            nc.vector.memset(ones_bf_row, 1.0)
            ones8 = consts.tile([128, 2, 16], F8, tag="ones8")
            nc.vector.memset(ones8, 1.0)
            ones_f_row = consts.tile([1, 128], F32, tag="ones_f_row")
            nc.vector.memset(ones_f_row, 1.0)

            # ---- load x / xr in strips, convert to bf16 (pipelined) ----
            STRIP = 1024
            xb = [xbp.tile([128, N], BF16, tag="xb", name="xb") for _ in range(2)]
            for s in range(N // STRIP):
                sl = slice(s * STRIP, (s + 1) * STRIP)
                for ci in range(2):
                    t = xfp.tile([128, STRIP], F32, tag="xf")
                    nc.gpsimd.dma_start(out=t, in_=xf_d[ci * 128 : (ci + 1) * 128, sl])
                    nc.vector.tensor_copy(xb[ci][:, sl], t)
            xr = [
                xrp.tile([128, ROWS], F32, tag="xr", name="xr") for _ in range(2)
            ]
            xrb = [
                xrbp.tile([128, ROWS], BF16, tag="xrb", name="xrb") for _ in range(2)
            ]
            for s in range(ROWS // STRIP):
                sl = slice(s * STRIP, (s + 1) * STRIP)
                for ci in range(2):
                    nc.gpsimd.dma_start(
                        out=xr[ci][:, sl], in_=xr_d[ci * 128 : (ci + 1) * 128, sl]
                    )
                    nc.vector.tensor_copy(xrb[ci][:, sl], xr[ci][:, sl])

            # ---- K = WkT.T @ X (+bk), Q = WqT.T @ XR (+bq) ----
            kbuf = kqp.tile([128, N], BF16, tag="kbuf")
            for nt in range(N // 512):
                ps = egp.tile([128, 512], F32, tag="eg")
                for ci in range(2):
                    nc.tensor.matmul(
                        ps,
                        wkT[:, ci, :],
                        xb[ci][:, nt * 512 : (nt + 1) * 512],
                        start=(ci == 0),
                        stop=(ci == 1),
                    )
                nc.vector.tensor_scalar_add(kbuf[:, nt * 512 : (nt + 1) * 512], ps, bk)
            qbuf = kqp.tile([128, ROWS], BF16, tag="qbuf")
            for nt in range(ROWS // 512):
                ps = egp.tile([128, 512], F32, tag="eg")
                for ci in range(2):
                    nc.tensor.matmul(
                        ps,
                        wqT[:, ci, :],
                        xrb[ci][:, nt * 512 : (nt + 1) * 512],
                        start=(ci == 0),
                        stop=(ci == 1),
                    )
                nc.vector.tensor_scalar_add(qbuf[:, nt * 512 : (nt + 1) * 512], ps, bq)

            # ---- VT[m, c] = X.T @ WvT + bv  (fp8 for DoubleRow PV) ----
            vt = vtp.tile([128, MB, C], F8, tag="vt")
            for mb in range(MB):
                ps = egp.tile([128, C], F32, tag="eg")
                for ci in range(2):
                    nc.tensor.matmul(
                        ps,
                        xb[ci][:, mb * 128 : (mb + 1) * 128],
                        wvT[:, ci, :],
                        start=(ci == 0),
                        stop=False,
                    )
                nc.tensor.matmul(ps, ones_bf_row, bvr, start=False, stop=True)
                nc.vector.tensor_copy(vt[:, mb, :], ps)

            # ---- attention main loop ----
            DR = mybir.MatmulPerfMode.DoubleRow
            for ch in range(NCH):
                qs = qbuf[:, ch * CHUNK : (ch + 1) * CHUNK]
                ptb = ptp.tile([128, MB, CHUNK], F8, tag="pt")
                u01 = [
                    upp.tile([128, CHUNK], F32, tag="u0", name="u0"),
                    upp.tile([128, CHUNK], F32, tag="u1", name="u1"),
                ]
                s_ps = spp.tile([16, CHUNK], F32, tag="s")
                for g in range(MB // 2):
                    eg = egp.tile([128, 2, CHUNK], F32, tag="eg")
                    for j in range(2):
                        mb = 2 * g + j
                        nc.tensor.matmul(
                            eg[:, j, :],
                            kbuf[:, mb * 128 : (mb + 1) * 128],
                            qs,
                            start=True,
                            stop=True,
                        )
                    nc.scalar.activation(
                        ptb[:, 2 * g : 2 * g + 2, :],
                        eg,
                        mybir.ActivationFunctionType.Exp,
                        scale=SCALE,
                    )
                    pair = ptb[:, 2 * g : 2 * g + 2, :]
                    # row sums S[n] += 1.P^T (fp8 DoubleRow, row 0 of 16)
                    nc.tensor.matmul(
                        s_ps,
                        ones8,
                        pair,
                        start=(g == 0),
                        stop=(g == MB // 2 - 1),
                        perf_mode=DR,
                    )
                    for cc in range(2):
                        nc.tensor.matmul(
                            u01[cc],
                            vt[:, 2 * g : 2 * g + 2, cc * 128 : (cc + 1) * 128],
                            pair,
                            start=(g == 0),
                            stop=(g == MB // 2 - 1),
                            perf_mode=DR,
                        )
                sinv = smp.tile([1, CHUNK], F32, tag="sinv")
                nc.vector.reciprocal(sinv, s_ps[0:1, :])
                sg = smp.tile([1, CHUNK], F32, tag="sg")
                nc.vector.tensor_scalar_mul(sg, sinv, gamma[0:1, 0:1])
                # broadcast gamma/S across partitions via k=1 matmul
                sgb_ps = bcp.tile([128, CHUNK], F32, tag="sgb")
                nc.tensor.matmul(sgb_ps, ones_f_row, sg, start=True, stop=True)
                sgb = smp.tile([128, CHUNK], F32, tag="sgbs")
                nc.vector.tensor_copy(sgb, sgb_ps)
                # y = (U * gamma/S) + 2*x
                for cc in range(2):
                    tmp = outp.tile([128, CHUNK], F32, tag="tmp")
                    nc.vector.tensor_tensor(tmp, u01[cc], sgb, op=mybir.AluOpType.mult)
                    out_t = outp.tile([128, CHUNK], F32, tag="out")
                    nc.vector.scalar_tensor_tensor(
                        out_t,
                        xr[cc][:, ch * CHUNK : (ch + 1) * CHUNK],
                        2.0,
                        tmp,
                        op0=mybir.AluOpType.mult,
                        op1=mybir.AluOpType.add,
                    )
                    nc.gpsimd.dma_start(
                        out=y_d[
                            cc * 128 : (cc + 1) * 128,
                            ch * CHUNK : (ch + 1) * CHUNK,
                        ],
                        in_=out_t,
                    )
    _split_waits(nc)
    return nc


_NC_CACHE = None


def _get_nc():
    global _NC_CACHE
    if _NC_CACHE is None:
        _NC_CACHE = _build()
    return _NC_CACHE


def kernel(x, Wq, bq, Wk, bk, Wv, bv, gamma):
    x = np.asarray(x, dtype=np.float32)
    nc = _get_nc()
    wqT = np.ascontiguousarray(np.asarray(Wq, np.float32).T.astype(NPBF16))
    wkT = np.ascontiguousarray(np.asarray(Wk, np.float32).T.astype(NPBF16))
    wvT = np.ascontiguousarray(np.asarray(Wv, np.float32).T.astype(NPBF16))
    shared = {
        "wqT": wqT,
        "wkT": wkT,
        "wvT": wvT,
        "bq": np.asarray(bq, np.float32).reshape(IC, 1).copy(),
        "bk": np.asarray(bk, np.float32).reshape(IC, 1).copy(),
        "bvr": np.asarray(bv, np.float32).reshape(1, C).astype(NPBF16).copy(),
        "gamma": np.asarray(gamma, np.float32).reshape(1, 1).copy(),
    }
    xflat = x.reshape(B, C, N)
    in_maps = []
    for core in range(NCORES):
        b, r = divmod(core, 2)
        xf = np.ascontiguousarray(xflat[b])
        xr = np.ascontiguousarray(xflat[b][:, r * ROWS : (r + 1) * ROWS])
        in_maps.append({"xf": xf, "xr": xr, **shared})

    trace = bool(int(os.environ.get("KERNEL_TRACE", "0")))
    res = run_bass_kernel_spmd(
        nc, in_maps, core_ids=list(range(NCORES)), trace=trace
    )
    if trace:
        global LAST_RESULT
        LAST_RESULT = res

    out = np.empty((B, C, N), np.float32)
    for core in range(NCORES):
        b, r = divmod(core, 2)
        out[b][:, r * ROWS : (r + 1) * ROWS] = res.results[core]["y"]
    return out.reshape(B, C, H, W)


if __name__ == "__main__":
    rng = np.random.default_rng(0)
    x = rng.standard_normal((B, C, H, W), dtype=np.float32)
    s = 0.02
    out = kernel(
        x=x,
        Wq=(rng.standard_normal((IC, C)) * s).astype(np.float32),
        bq=np.zeros(IC, np.float32),
        Wk=(rng.standard_normal((IC, C)) * s).astype(np.float32),
        bk=np.zeros(IC, np.float32),
        Wv=(rng.standard_normal((C, C)) * s).astype(np.float32),
        bv=np.zeros(C, np.float32),
        gamma=np.full(1, 0.1, np.float32),
    )
    print("out", out.shape, out.dtype, float(out.ravel()[0]))
